# revision 39
# baseline (speedup 1.0000x reference)
"""GCLSTM (Chebyshev K=3 graph-conv LSTM gates) forward on 8 Trainium2 NeuronCores.

Math (derived from the reference model): the scan carry is unused and H/C start
at zero inside each step, so the output depends only on the LAST timestep and
every _cheb(H, ...) term reduces to its bias. What remains per output row i:

    deg[i]  = sum_{e: row[e]=i} w[e]
    dis     = deg > 0 ? 1/sqrt(max(deg, 1e-30)) : 0
    U1      = sum_{e: row[e]=i} (w[e]*dis[col[e]]) * X[col[e]]   (dis_col in mt)
    Tx1     = -dis * U1
    U2      = S(dis^2 * U1)      with plain w weights
    G_g     = X@(W[g,0]-W[g,2]) + Tx1@W[g,1] + (2*dis*U2)@W[g,2] + bias_g
    I = sigmoid(G_i); Tt = tanh(G_c); C = I*Tt
    O = sigmoid(G_o + wc[2]*C);  out = relu(O * tanh(C))

Sharding: nodes are 1-D partitioned across the 8 cores (rows of the
segment-sum stay local). The SpMM-1 gather table is just fp16(X) scaled into
the one-hot weights (dis[col] folded into mt1), so it is staged host-side on
every core and needs NO collective; only the SpMM-2 table (dis^2*U1, runtime
data) is exchanged — as TWO sub-range AllGathers so the first fires halfway
through SpMM-1 and hides the collective latency entirely.

The per-edge scatter-add is a dense matmul against host-built one-hot
"staircase" matrices in fp8-e4m3 (rel err ~6e-3, tolerance 2e-2), with edges
(bucketed by (row-block, col-half)) as the contraction dim; the per-edge
gather uses the SWDGE dma_gather custom instruction (int16 indices, hence the
col-half split — halves = node sub-ranges so they align with the AllGather
split). Gates run feature-major in bf16 (X arrives pre-transposed via an xbar
transpose-DMA; biases ride the scalar-engine activations), and all per-block
PE work that depends on cross-engine chains (Tx1/B transposes, gate matmuls,
output transposes) is software-pipelined one block behind the SpMM matmuls so
the tensor engine never stalls the gather stream.
"""

import ml_dtypes
import numpy as np

P = 128
NCORES = 8
EDGE_NP = np.float16            # gather-table dtype
MT_NP = ml_dtypes.float8_e4m3   # one-hot scatter-matrix dtype
BF16 = ml_dtypes.bfloat16
SWDGE_SCRATCH = 16384   # descriptor-ring carveout (ring limit is fixed at 1024 descs)
CALL_G = 8              # groups per dma_gather call (ring limit 1024 idxs)

# ----------------------------------------------------------------------------
# Host-side sharding / bucketing
# ----------------------------------------------------------------------------


def _preprocess(X, row, col, w):
    """Bucket edges by (owner core, row block, col sub-range); build inputs."""
    N, F = X.shape
    assert F == P
    R = -(-N // NCORES)              # rows owned per core
    RB = -(-R // P)                  # 128-row blocks per core
    R_PAD = RB * P
    NFULL = NCORES * R_PAD
    S0B = (RB + 1) // 2              # blocks in sub-range 0 (per core)
    S1B = RB - S0B
    R0, R1 = S0B * P, S1B * P
    HALFS = (NCORES * R0, NCORES * R1)   # gather-table half sizes
    assert max(HALFS) <= 32768, "int16 gather index limit"

    core = (row // R).astype(np.int64)
    lrow = (row - core * R).astype(np.int64)          # 0..R-1

    colc = col // R
    clr = col - colc * R                              # col's local row
    sub = (clr >= R0).astype(np.int64)
    # index within the half's table
    cidx_h = np.where(sub == 0, colc * R0 + clr, colc * R1 + (clr - R0))

    blk = lrow // P                                   # row block 0..RB-1
    key = sub * RB + blk                              # half-major segment order

    deg = np.bincount(row, weights=w, minlength=N).astype(np.float32)
    dis_full = np.where(deg > 0,
                        1.0 / np.sqrt(np.maximum(deg, 1e-30)), 0.0).astype(np.float32)
    wdis = (w * dis_full[col]).astype(np.float32)     # SpMM-1 scatter weights

    # Dedupe edges sharing (segment, col): one gathered slot can feed many
    # destination rows (the scatter-matrix column is multi-hot), so the
    # descriptor count per segment is the number of UNIQUE cols, and slots
    # come out col-sorted (monotonic HBM addresses). Two passes: unique
    # counts fix the shared group layout G, then per-core slot assignment.
    SEGS = 2 * RB
    per_core = []
    ucnt = np.zeros((NCORES, SEGS), np.int64)
    for c in range(NCORES):
        sel = core == c
        codes = key[sel] * 32768 + cidx_h[sel]
        uc, inv = np.unique(codes, return_inverse=True)
        per_core.append((sel, uc, inv))
        cseg = np.bincount(uc >> 15, minlength=SEGS)
        ucnt[c] = cseg
    gseg = ucnt.max(axis=0)                           # max unique per segment
    G = np.maximum(1, -(-gseg.reshape(2, RB).T // P))  # [RB, 2] groups, >=1
    Lseg = np.ascontiguousarray(G.T) * P              # [2, RB] padded slots
    seg_start = np.concatenate([[0], np.cumsum(Lseg.ravel())])[:-1]
    TOT = int(Lseg.sum())                             # padded slots per core
    TG = TOT // P                                     # total groups per core

    # replicated fp16(X) gather table for SpMM-1, in (sub, core, lr) layout
    x_tab = np.zeros((sum(HALFS), P), EDGE_NP)
    for c in range(NCORES):
        lo, hi = c * R, min((c + 1) * R, N)
        xl = np.zeros((R_PAD, P), EDGE_NP)
        xl[: hi - lo] = X[lo:hi].astype(EDGE_NP)
        x_tab[c * R0:(c + 1) * R0] = xl[:R0]
        x_tab[HALFS[0] + c * R1:HALFS[0] + (c + 1) * R1] = xl[R0:]

    Lflat = Lseg.ravel()
    in_maps = []
    for c in range(NCORES):
        sel, uc, inv = per_core[c]
        useg = uc >> 15                               # segment of each unique
        uch = uc & 32767                              # col-in-half
        cseg = np.bincount(useg, minlength=SEGS)
        starts = np.concatenate([[0], np.cumsum(cseg)])[:-1]
        rank = np.arange(len(uc)) - starts[useg]
        upos = seg_start[useg] + rank                 # slot of each unique

        colp_arr = np.zeros(TOT, np.int64)
        colp_arr[upos] = uch
        # padding slots repeat the segment's last real address (row-buffer
        # hits make them nearly free); empty segments keep idx 0
        for s in range(SEGS):
            lo_s = seg_start[s] + cseg[s]
            hi_s = seg_start[s] + Lflat[s]
            if lo_s < hi_s and cseg[s] > 0:
                colp_arr[lo_s:hi_s] = colp_arr[lo_s - 1]

        lr_c = lrow[sel] % P
        slot = upos[inv]                              # slot of each edge
        mcol = (slot // P) * P + lr_c
        mt_f32 = np.zeros((P, 2 * TG * P), np.float32)
        np.add.at(mt_f32, (slot % P, mcol), wdis[sel])
        np.add.at(mt_f32, (slot % P, TG * P + mcol), w[sel])
        mt_all = mt_f32.astype(MT_NP)
        del mt_f32

        idx16 = colp_arr.reshape(-1, 16).T            # [16, TOT/16]
        idx_all = np.tile(idx16, (8, 1)).astype(np.int16)

        lo, hi = c * R, min((c + 1) * R, N)
        dpad = np.zeros(R_PAD, np.float32)
        dpad[: hi - lo] = dis_full[lo:hi]
        dis_t = np.ascontiguousarray(dpad.reshape(RB, P).T)   # [P, RB]

        x_bf = np.zeros((R_PAD, P), BF16)
        x_bf[: hi - lo] = X[lo:hi].astype(BF16)

        in_maps.append(dict(idx_all=idx_all, mt_all=mt_all,
                            dis_t=dis_t, x_bf=x_bf, x_tab=x_tab))

    cfg = dict(N=N, R=R, RB=RB, R_PAD=R_PAD, S0B=S0B, S1B=S1B,
               HALFS=HALFS, TG=TG, G=G)
    return in_maps, cfg


# ----------------------------------------------------------------------------
# Device kernel
# ----------------------------------------------------------------------------


def _build(cfg):
    import concourse.bacc as bacc
    import concourse.mybir as mybir
    import concourse.tile as tile

    RB, TG = cfg["RB"], cfg["TG"]
    R_PAD, HALFS = cfg["R_PAD"], cfg["HALFS"]
    S0B, S1B = cfg["S0B"], cfg["S1B"]
    G = cfg["G"]
    f32 = mybir.dt.float32
    edt = mybir.dt.float16
    mdt = mybir.dt.float8e4
    bdt = mybir.dt.bfloat16
    Alu = mybir.AluOpType
    Act = mybir.ActivationFunctionType
    NG = 3  # gates i, c, o
    NTAB = HALFS[0] + HALFS[1]

    nc = bacc.Bacc("TRN2", target_bir_lowering=False, debug=False,
                   num_devices=NCORES, num_swdge_queues=4,
                   dynamic_dma_scratch_size=SWDGE_SCRATCH)

    x_tab = nc.dram_tensor("x_tab", [NTAB, P], edt, kind="ExternalInput")
    x_bf = nc.dram_tensor("x_bf", [R_PAD, P], bdt, kind="ExternalInput")
    dis_in = nc.dram_tensor("dis_t", [P, RB], f32, kind="ExternalInput")
    idx_all = nc.dram_tensor("idx_all", [P, TG * 8], mybir.dt.int16, kind="ExternalInput")
    mt_all = nc.dram_tensor("mt_all", [P, 2 * TG * P], mdt, kind="ExternalInput")
    wmats = nc.dram_tensor("wmats", [NG, 3, P, P], bdt, kind="ExternalInput")
    ident_in = nc.dram_tensor("ident_in", [P, P], f32, kind="ExternalInput")
    bias_c = nc.dram_tensor("bias_c", [P, NG], f32, kind="ExternalInput")
    wc2_c = nc.dram_tensor("wc2_c", [P, 1], f32, kind="ExternalInput")
    out_loc = nc.dram_tensor("out_loc", [R_PAD, P], f32, kind="ExternalOutput")

    out_r = out_loc.rearrange("(b p) f -> p b f", p=P)

    with tile.TileContext(nc) as tc:
        with (
            tc.tile_pool(name="const", bufs=1) as const,
            tc.tile_pool(name="pers", bufs=1) as pers,
            tc.tile_pool(name="work", bufs=3) as work,
            tc.tile_pool(name="vpool", bufs=20) as vpool,
            tc.tile_pool(name="mtpool", bufs=7) as mtpool,
            tc.tile_pool(name="ppool", bufs=2, space="PSUM") as ppool,
            tc.tile_pool(name="tpsum", bufs=2, space="PSUM") as tpsum,
            tc.tile_pool(name="gpsum", bufs=3, space="PSUM") as gpsum,
            tc.tile_pool(name="xpsum", bufs=1, space="PSUM") as xpsum,
            tc.tile_pool(name="dram", bufs=1, space="DRAM") as dram,
        ):
            # ---------------- latency-critical input DMAs first ----------
            GH = [int(G[:, 0].sum()), int(G[:, 1].sum())]
            # per half: a small head tile (first A_CALLS gather calls) so the
            # first gather fires without waiting for the full index load
            A_COLS = 4 * CALL_G * 8           # 4 calls worth of idx columns
            idx_sb = []
            for h in (0, 1):
                off = 0 if h == 0 else GH[0] * 8
                ca = min(A_COLS, GH[h] * 8)
                ta = pers.tile([P, ca], mybir.dt.int16, tag=f"idxa{h}")
                nc.scalar.dma_start(out=ta[:], in_=idx_all[:, off:off + ca])
                tb = None
                if GH[h] * 8 > ca:
                    tb = pers.tile([P, GH[h] * 8 - ca], mybir.dt.int16,
                                   tag=f"idxb{h}")
                    nc.scalar.dma_start(out=tb[:],
                                        in_=idx_all[:, off + ca:off + GH[h] * 8])
                idx_sb.append((ta, tb, ca))
            dis = const.tile([P, RB], f32)
            nc.sync.dma_start(out=dis[:], in_=dis_in[:])

            dis2 = const.tile([P, RB], f32)
            nc.vector.tensor_tensor(out=dis2[:], in0=dis[:], in1=dis[:], op=Alu.mult)
            ndis = const.tile([P, RB], f32)
            nc.vector.tensor_scalar(out=ndis[:], in0=dis[:], scalar1=-1.0,
                                    scalar2=None, op0=Alu.mult)
            dis2x = const.tile([P, RB], f32)
            nc.vector.tensor_scalar(out=dis2x[:], in0=dis[:], scalar1=2.0,
                                    scalar2=None, op0=Alu.mult)

            # ---------------- constants ----------------
            ident = const.tile([P, P], f32)
            nc.sync.dma_start(out=ident[:], in_=ident_in[:])
            ident_b = const.tile([P, P], bdt)
            nc.scalar.copy(out=ident_b[:], in_=ident[:])
            wsb = {}
            for gi in range(NG):
                for k in range(3):
                    t = const.tile([P, P], bdt, tag=f"w{gi}{k}")
                    nc.sync.dma_start(out=t[:], in_=wmats[gi, k])
                    wsb[(gi, k)] = t
            bias_sb = const.tile([P, NG], f32)
            nc.sync.dma_start(out=bias_sb[:], in_=bias_c[:])
            wc2_sb = const.tile([P, 1], f32)
            nc.sync.dma_start(out=wc2_sb[:], in_=wc2_c[:])

            # shared SpMM: per half, one contiguous run of gather calls
            # (CALL_G*128 idxs each, SWDGE ring limit) decoupled from block
            # boundaries; per (block, half) a one-hot matmul chain into PSUM.
            qctr = [0]
            nreg_cache = {}

            def nreg(n):
                if n not in nreg_cache:
                    nreg_cache[n] = nc.gpsimd.to_reg(n)
                return nreg_cache[n]

            cumG = np.concatenate([np.zeros((1, 2), np.int64),
                                   np.cumsum(G, axis=0)], axis=0)  # [RB+1, 2]

            def spmm(srcs, moff, consume):
                for h in (0, 1):
                    hoff = 0 if h == 0 else GH[0]
                    nh = GH[h]
                    src_ap = srcs[h]
                    vt = {}
                    emitted = [-1]

                    def ensure_call(k, h=h, hoff=hoff, nh=nh, src_ap=src_ap,
                                    vt=vt, emitted=emitted):
                        while emitted[0] < k:
                            kk = emitted[0] + 1
                            gc = min(CALL_G, nh - kk * CALL_G)
                            c0 = kk * CALL_G * 8          # idx column offset
                            ta, tb, ca = idx_sb[h]
                            if c0 < ca:
                                iap = ta[:, c0:c0 + gc * 8]
                            else:
                                iap = tb[:, c0 - ca:c0 - ca + gc * 8]
                            v = vpool.tile([P, CALL_G, P], edt, tag="v",
                                           name=f"v_{h}_{kk}")
                            nc.gpsimd.dma_gather(
                                out_ap=v[:, :gc, :],
                                in_ap=src_ap,
                                idxs_ap=iap,
                                num_idxs=gc * P, num_idxs_reg=nreg(gc * P),
                                elem_size=P, queue_num=qctr[0] % 4,
                                single_packet=False)
                            qctr[0] += 1
                            vt[kk] = v
                            vt.pop(kk - 20, None)
                            emitted[0] = kk
                    for b in range(RB):
                        s_b, e_b = int(cumG[b, h]), int(cumG[b + 1, h])
                        gs = e_b - s_b
                        goff = hoff + s_b
                        mt = mtpool.tile([P, int(G.max()) * P], mdt, tag="mt")
                        nc.sync.dma_start(
                            out=mt[:, :gs * P],
                            in_=mt_all[:, moff + goff * P:moff + (goff + gs) * P])
                        ps = ppool.tile([P, P], f32, tag="u", name=f"ps_{h}_{b}")
                        for gl_ in range(s_b, e_b):
                            k = gl_ // CALL_G
                            ensure_call(min(k + 6, (nh - 1) // CALL_G))
                            nc.tensor.matmul(
                                out=ps[:], lhsT=mt[:, (gl_ - s_b) * P:(gl_ - s_b + 1) * P],
                                rhs=vt[k][:, gl_ % CALL_G, :],
                                start=(gl_ == s_b), stop=(h == 0 and gl_ == e_b - 1))
                        if h == 1:
                            # fold the h0 partial into the PSUM chain on the
                            # PE (the vector engine is port-starved here)
                            nc.tensor.matmul(out=ps[:], lhsT=ident[:],
                                             rhs=usb[:, b, :],
                                             start=False, stop=True)
                        consume(b, ps, h)

            # ---------------- SpMM 1 (gathers straight from x_tab) --------
            usb = pers.tile([P, RB, P], f32, tag="usb")     # h0 scratch, reused
            u1t = pers.tile([P, RB, P], bdt, tag="u1t")     # Tx1, feature-major
            y2sb = pers.tile([P, RB, P], edt, tag="y2sb")   # dis^2*U1 staging
            # X^T built by per-block PE transposes during SpMM-1 (an xbar
            # transpose-DMA would serialize against the gather DMAs)
            xn_sb = pers.tile([P, RB, P], bdt, tag="xn")
            nc.sync.dma_start(out=xn_sb[:],
                              in_=x_bf.rearrange("(b p) f -> p b f", p=P))
            xt_sb = pers.tile([P, RB, P], bdt, tag="xt")

            y2f = [dram.tile([HALFS[0], P], edt, addr_space="Shared",
                             name="y2f0"),
                   dram.tile([HALFS[1], P], edt, addr_space="Shared",
                             name="y2f1")]
            y2ag_in = [dram.tile([S0B * P, P], edt, name="y2ag0"),
                       dram.tile([S1B * P, P], edt, name="y2ag1")]

            def fire_ag_dma(s):
                b0 = 0 if s == 0 else S0B
                nb = S0B if s == 0 else S1B
                nc.sync.dma_start(
                    out=y2ag_in[s][:].rearrange("(b p) f -> p b f", p=P),
                    in_=y2sb[:, b0:b0 + nb, :])

            def fire_ag_coll(s):
                nc.gpsimd.collective_compute(
                    "AllGather", Alu.bypass,
                    replica_groups=[list(range(NCORES))],
                    ins=[y2ag_in[s].opt()], outs=[y2f[s].opt()])

            # software pipeline state: per-block tiles finished one block late
            pend1 = {}

            def flush1():
                if not pend1:
                    return
                b, at = pend1.popitem()
                tp = tpsum.tile([P, P], f32, tag="tp", space="PSUM")
                nc.tensor.transpose(out=tp[:], in_=at[:], identity=ident[:])
                nc.scalar.copy(out=u1t[:, b, :], in_=tp[:])

            def consume1(b, ps, h):
                if h == 0:
                    tp = xpsum.tile([P, P], bdt, tag="tpx", space="PSUM")
                    nc.tensor.transpose(out=tp[:], in_=xn_sb[:, b, :],
                                        identity=ident_b[:])
                    nc.scalar.copy(out=xt_sb[:, b, :], in_=tp[:])
                    nc.scalar.copy(out=usb[:, b, :], in_=ps[:])
                    return
                nc.scalar.activation(out=y2sb[:, b, :], in_=ps[:], func=Act.Copy,
                                     scale=dis2[:, b:b + 1])
                at = work.tile([P, P], f32, tag="at")
                nc.scalar.activation(out=at[:], in_=ps[:], func=Act.Copy,
                                     scale=ndis[:, b:b + 1])
                flush1()
                pend1[b] = at
                if b == S0B - 1:
                    fire_ag_dma(0)
                elif b == S0B + 3:
                    fire_ag_coll(0)

            spmm((x_tab[0:HALFS[0], :], x_tab[HALFS[0]:NTAB, :]), 0, consume1)
            flush1()
            fire_ag_dma(1)

            # ---------------- SpMM 2 + gates (feature-major), fused -------
            # three-stage software pipeline so every PE item's inputs are at
            # least one full block old: block b's V-ops now; B-transpose for
            # b-1; gate matmuls + activations for b-2; output store for b-3.
            pend_tp = {}
            pend_g = {}
            pend_o = {}

            def flush_out():
                if not pend_o:
                    return
                b, res = pend_o.popitem()
                tpo = tpsum.tile([P, P], f32, tag="tp", space="PSUM")
                nc.tensor.transpose(out=tpo[:], in_=res[:], identity=ident[:])
                res_nm = work.tile([P, P], f32, tag="rnm")
                nc.scalar.copy(out=res_nm[:], in_=tpo[:])
                nc.sync.dma_start(out=out_r[:, b, :], in_=res_nm[:])

            def flush_tp():
                if not pend_tp:
                    return
                b, btmp = pend_tp.popitem()
                tpb = tpsum.tile([P, P], f32, tag="tp", space="PSUM")
                nc.tensor.transpose(out=tpb[:], in_=btmp[:], identity=ident[:])
                btm = work.tile([P, P], bdt, tag="btm")
                nc.scalar.copy(out=btm[:], in_=tpb[:])
                pend_g[b] = btm

            def flush_gates():
                if not pend_g:
                    return
                b, btm = pend_g.popitem()
                gate_ps = []
                for gi in range(NG):
                    pg = gpsum.tile([P, P], f32, tag="g", space="PSUM")
                    nc.tensor.matmul(out=pg[:], lhsT=wsb[(gi, 0)][:],
                                     rhs=xt_sb[:, b, :], start=True, stop=False)
                    nc.tensor.matmul(out=pg[:], lhsT=wsb[(gi, 1)][:],
                                     rhs=u1t[:, b, :], start=False, stop=False)
                    nc.tensor.matmul(out=pg[:], lhsT=wsb[(gi, 2)][:],
                                     rhs=btm[:], start=False, stop=True)
                    gate_ps.append(pg)
                i_t = work.tile([P, P], f32, tag="i")
                nc.scalar.activation(out=i_t[:], in_=gate_ps[0][:], func=Act.Sigmoid,
                                     bias=bias_sb[:, 0:1])
                tt_t = work.tile([P, P], f32, tag="tt")
                nc.scalar.activation(out=tt_t[:], in_=gate_ps[1][:], func=Act.Tanh,
                                     bias=bias_sb[:, 1:2])
                c_t = work.tile([P, P], f32, tag="c")
                nc.vector.tensor_tensor(out=c_t[:], in0=i_t[:], in1=tt_t[:],
                                        op=Alu.mult)
                oin = work.tile([P, P], f32, tag="oin")
                nc.scalar.activation(out=oin[:], in_=c_t[:], func=Act.Copy,
                                     scale=wc2_sb[:, 0:1])
                oin2 = work.tile([P, P], f32, tag="oin2")
                nc.vector.tensor_tensor(out=oin2[:], in0=gate_ps[2][:], in1=oin[:],
                                        op=Alu.add)
                o_t = work.tile([P, P], f32, tag="o")
                nc.scalar.activation(out=o_t[:], in_=oin2[:], func=Act.Sigmoid,
                                     bias=bias_sb[:, 2:3])
                tc_t = work.tile([P, P], f32, tag="tc")
                nc.scalar.activation(out=tc_t[:], in_=c_t[:], func=Act.Tanh)
                h_t = work.tile([P, P], f32, tag="h")
                nc.vector.tensor_tensor(out=h_t[:], in0=o_t[:], in1=tc_t[:],
                                        op=Alu.mult)
                res = work.tile([P, P], f32, tag="res")
                nc.scalar.activation(out=res[:], in_=h_t[:], func=Act.Relu)
                pend_o[b] = res

            def consume2(b, ps2, h):
                if h == 0:
                    if b == 2:
                        fire_ag_coll(1)
                    nc.scalar.copy(out=usb[:, b, :], in_=ps2[:])
                    return
                btmp = work.tile([P, P], f32, tag="btmp")
                nc.scalar.activation(out=btmp[:], in_=ps2[:], func=Act.Copy,
                                     scale=dis2x[:, b:b + 1])
                flush_out()
                flush_gates()
                flush_tp()
                pend_tp[b] = btmp

            spmm((y2f[0][:], y2f[1][:]), TG * P, consume2)
            while pend_tp or pend_g or pend_o:
                flush_out()
                flush_tp()
                flush_gates()

    nc.compile()
    return nc


# ----------------------------------------------------------------------------
# Entry point
# ----------------------------------------------------------------------------

_CACHE = {}


def _get_built(cfg_key, cfg):
    if cfg_key not in _CACHE:
        _CACHE[cfg_key] = _build(cfg)
    return _CACHE[cfg_key]


def _make_in_maps(inputs):
    node_feats = np.asarray(inputs["node_feats"])
    edge_feats = np.asarray(inputs["edge_feats"], np.float32)
    edge_index = np.asarray(inputs["edge_index"])
    t = node_feats.shape[0] - 1
    X = np.asarray(node_feats[t], np.float32)
    row = np.asarray(edge_index[t, 0], np.int64)
    col = np.asarray(edge_index[t, 1], np.int64)
    w = np.asarray(edge_feats[t], np.float32)

    in_maps, cfg = _preprocess(X, row, col, w)

    Wx = np.asarray(inputs["Wx"], np.float32)
    bsum = (np.asarray(inputs["bx"], np.float32)
            + np.asarray(inputs["bh"], np.float32)
            + np.asarray(inputs["bg"], np.float32))          # [4, FH]
    wc = np.asarray(inputs["wc"], np.float32)                # [3, FH]
    GATES = (0, 2, 3)
    wm = np.empty((3, 3, P, P), BF16)
    for gi, g in enumerate(GATES):
        wm[gi, 0] = (Wx[g, 0] - Wx[g, 2]).astype(BF16)
        wm[gi, 1] = Wx[g, 1].astype(BF16)
        wm[gi, 2] = Wx[g, 2].astype(BF16)
    bias_c = np.ascontiguousarray(bsum[list(GATES)].T)       # [P, 3]
    wc2_c = np.ascontiguousarray(wc[2].reshape(P, 1))        # [P, 1]
    ident = np.eye(P, dtype=np.float32)
    for m in in_maps:
        m["wmats"] = wm
        m["bias_c"] = bias_c
        m["wc2_c"] = wc2_c
        m["ident_in"] = ident
    return in_maps, cfg


def _run(inputs, trace=False):
    from concourse.bass_utils import run_bass_kernel_spmd

    in_maps, cfg = _make_in_maps(inputs)
    key = (cfg["N"], cfg["RB"], cfg["TG"],
           tuple(cfg["G"].ravel().tolist()))
    nc = _get_built(key, cfg)
    res = run_bass_kernel_spmd(nc, in_maps, core_ids=list(range(NCORES)),
                               trace=trace)
    N, R, R_PAD = cfg["N"], cfg["R"], cfg["R_PAD"]
    out = np.empty((N, P), np.float32)
    for c in range(NCORES):
        lo, hi = c * R, min((c + 1) * R, N)
        out[lo:hi] = res.results[c]["out_loc"][: hi - lo]
    return out, res.exec_time_ns


def kernel(**inputs) -> np.ndarray:
    out, _ = _run(inputs, trace=False)
    return out


# revision 41
# speedup vs baseline: 1.0001x; 1.0001x over previous
"""GCLSTM (Chebyshev K=3 graph-conv LSTM gates) forward on 8 Trainium2 NeuronCores.

Math (derived from the reference model): the scan carry is unused and H/C start
at zero inside each step, so the output depends only on the LAST timestep and
every _cheb(H, ...) term reduces to its bias. What remains per output row i:

    deg[i]  = sum_{e: row[e]=i} w[e]
    dis     = deg > 0 ? 1/sqrt(max(deg, 1e-30)) : 0
    U1      = sum_{e: row[e]=i} (w[e]*dis[col[e]]) * X[col[e]]   (dis_col in mt)
    Tx1     = -dis * U1
    U2      = S(dis^2 * U1)      with plain w weights
    G_g     = X@(W[g,0]-W[g,2]) + Tx1@W[g,1] + (2*dis*U2)@W[g,2] + bias_g
    I = sigmoid(G_i); Tt = tanh(G_c); C = I*Tt
    O = sigmoid(G_o + wc[2]*C);  out = relu(O * tanh(C))

Sharding: nodes are 1-D partitioned across the 8 cores (rows of the
segment-sum stay local). The SpMM-1 gather table is just fp16(X) scaled into
the one-hot weights (dis[col] folded into mt1), so it is staged host-side on
every core and needs NO collective; only the SpMM-2 table (dis^2*U1, runtime
data) is exchanged — as TWO sub-range AllGathers so the first fires halfway
through SpMM-1 and hides the collective latency entirely.

The per-edge scatter-add is a dense matmul against host-built one-hot
"staircase" matrices in fp8-e4m3 (rel err ~6e-3, tolerance 2e-2), with edges
(bucketed by (row-block, col-half)) as the contraction dim; the per-edge
gather uses the SWDGE dma_gather custom instruction (int16 indices, hence the
col-half split — halves = node sub-ranges so they align with the AllGather
split). Gates run feature-major in bf16 (X arrives pre-transposed via an xbar
transpose-DMA; biases ride the scalar-engine activations), and all per-block
PE work that depends on cross-engine chains (Tx1/B transposes, gate matmuls,
output transposes) is software-pipelined one block behind the SpMM matmuls so
the tensor engine never stalls the gather stream.
"""

import ml_dtypes
import numpy as np

P = 128
NCORES = 8
EDGE_NP = np.float16            # gather-table dtype
MT_NP = ml_dtypes.float8_e4m3   # one-hot scatter-matrix dtype
BF16 = ml_dtypes.bfloat16
SWDGE_SCRATCH = 16384   # descriptor-ring carveout (ring limit is fixed at 1024 descs)
CALL_G = 8              # groups per dma_gather call (ring limit 1024 idxs)

# ----------------------------------------------------------------------------
# Host-side sharding / bucketing
# ----------------------------------------------------------------------------


def _preprocess(X, row, col, w):
    """Bucket edges by (owner core, row block, col sub-range); build inputs."""
    N, F = X.shape
    assert F == P
    R = -(-N // NCORES)              # rows owned per core
    RB = -(-R // P)                  # 128-row blocks per core
    R_PAD = RB * P
    NFULL = NCORES * R_PAD
    S0B = (RB + 1) // 2              # blocks in sub-range 0 (per core)
    S1B = RB - S0B
    R0, R1 = S0B * P, S1B * P
    HALFS = (NCORES * R0, NCORES * R1)   # gather-table half sizes
    assert max(HALFS) <= 32768, "int16 gather index limit"

    core = (row // R).astype(np.int64)
    lrow = (row - core * R).astype(np.int64)          # 0..R-1

    colc = col // R
    clr = col - colc * R                              # col's local row
    sub = (clr >= R0).astype(np.int64)
    # index within the half's table
    cidx_h = np.where(sub == 0, colc * R0 + clr, colc * R1 + (clr - R0))

    blk = lrow // P                                   # row block 0..RB-1
    key = sub * RB + blk                              # half-major segment order

    deg = np.bincount(row, weights=w, minlength=N).astype(np.float32)
    dis_full = np.where(deg > 0,
                        1.0 / np.sqrt(np.maximum(deg, 1e-30)), 0.0).astype(np.float32)
    wdis = (w * dis_full[col]).astype(np.float32)     # SpMM-1 scatter weights

    # Dedupe edges sharing (segment, col): one gathered slot can feed many
    # destination rows (the scatter-matrix column is multi-hot), so the
    # descriptor count per segment is the number of UNIQUE cols, and slots
    # come out col-sorted (monotonic HBM addresses). Two passes: unique
    # counts fix the shared group layout G, then per-core slot assignment.
    SEGS = 2 * RB
    per_core = []
    ucnt = np.zeros((NCORES, SEGS), np.int64)
    for c in range(NCORES):
        sel = core == c
        codes = key[sel] * 32768 + cidx_h[sel]
        uc, inv = np.unique(codes, return_inverse=True)
        per_core.append((sel, uc, inv))
        cseg = np.bincount(uc >> 15, minlength=SEGS)
        ucnt[c] = cseg
    gseg = ucnt.max(axis=0)                           # max unique per segment
    G = np.maximum(1, -(-gseg.reshape(2, RB).T // P))  # [RB, 2] groups, >=1
    Lseg = np.ascontiguousarray(G.T) * P              # [2, RB] padded slots
    seg_start = np.concatenate([[0], np.cumsum(Lseg.ravel())])[:-1]
    TOT = int(Lseg.sum())                             # padded slots per core
    TG = TOT // P                                     # total groups per core

    # replicated fp16(X) gather table for SpMM-1, in (sub, core, lr) layout
    x_tab = np.zeros((sum(HALFS), P), EDGE_NP)
    for c in range(NCORES):
        lo, hi = c * R, min((c + 1) * R, N)
        xl = np.zeros((R_PAD, P), EDGE_NP)
        xl[: hi - lo] = X[lo:hi].astype(EDGE_NP)
        x_tab[c * R0:(c + 1) * R0] = xl[:R0]
        x_tab[HALFS[0] + c * R1:HALFS[0] + (c + 1) * R1] = xl[R0:]

    Lflat = Lseg.ravel()
    in_maps = []
    for c in range(NCORES):
        sel, uc, inv = per_core[c]
        useg = uc >> 15                               # segment of each unique
        uch = uc & 32767                              # col-in-half
        cseg = np.bincount(useg, minlength=SEGS)
        starts = np.concatenate([[0], np.cumsum(cseg)])[:-1]
        rank = np.arange(len(uc)) - starts[useg]
        upos = seg_start[useg] + rank                 # slot of each unique

        colp_arr = np.zeros(TOT, np.int64)
        colp_arr[upos] = uch
        # padding slots repeat the segment's last real address (row-buffer
        # hits make them nearly free); empty segments keep idx 0
        for s in range(SEGS):
            lo_s = seg_start[s] + cseg[s]
            hi_s = seg_start[s] + Lflat[s]
            if lo_s < hi_s and cseg[s] > 0:
                colp_arr[lo_s:hi_s] = colp_arr[lo_s - 1]

        lr_c = lrow[sel] % P
        slot = upos[inv]                              # slot of each edge
        mcol = (slot // P) * P + lr_c
        mt_f32 = np.zeros((P, 2 * TG * P), np.float32)
        np.add.at(mt_f32, (slot % P, mcol), wdis[sel])
        np.add.at(mt_f32, (slot % P, TG * P + mcol), w[sel])
        mt_all = mt_f32.astype(MT_NP)
        del mt_f32

        idx16 = colp_arr.reshape(-1, 16).T            # [16, TOT/16]
        idx_all = np.tile(idx16, (8, 1)).astype(np.int16)

        lo, hi = c * R, min((c + 1) * R, N)
        dpad = np.zeros(R_PAD, np.float32)
        dpad[: hi - lo] = dis_full[lo:hi]
        dis_t = np.ascontiguousarray(dpad.reshape(RB, P).T)   # [P, RB]

        x_bf = np.zeros((R_PAD, P), BF16)
        x_bf[: hi - lo] = X[lo:hi].astype(BF16)

        in_maps.append(dict(idx_all=idx_all, mt_all=mt_all,
                            dis_t=dis_t, x_bf=x_bf, x_tab=x_tab))

    cfg = dict(N=N, R=R, RB=RB, R_PAD=R_PAD, S0B=S0B, S1B=S1B,
               HALFS=HALFS, TG=TG, G=G)
    return in_maps, cfg


# ----------------------------------------------------------------------------
# Device kernel
# ----------------------------------------------------------------------------


def _build(cfg):
    import concourse.bacc as bacc
    import concourse.mybir as mybir
    import concourse.tile as tile

    RB, TG = cfg["RB"], cfg["TG"]
    R_PAD, HALFS = cfg["R_PAD"], cfg["HALFS"]
    S0B, S1B = cfg["S0B"], cfg["S1B"]
    G = cfg["G"]
    f32 = mybir.dt.float32
    edt = mybir.dt.float16
    mdt = mybir.dt.float8e4
    bdt = mybir.dt.bfloat16
    Alu = mybir.AluOpType
    Act = mybir.ActivationFunctionType
    NG = 3  # gates i, c, o
    NTAB = HALFS[0] + HALFS[1]

    nc = bacc.Bacc("TRN2", target_bir_lowering=False, debug=False,
                   num_devices=NCORES, num_swdge_queues=4,
                   dynamic_dma_scratch_size=SWDGE_SCRATCH)

    x_tab = nc.dram_tensor("x_tab", [NTAB, P], edt, kind="ExternalInput")
    x_bf = nc.dram_tensor("x_bf", [R_PAD, P], bdt, kind="ExternalInput")
    dis_in = nc.dram_tensor("dis_t", [P, RB], f32, kind="ExternalInput")
    idx_all = nc.dram_tensor("idx_all", [P, TG * 8], mybir.dt.int16, kind="ExternalInput")
    mt_all = nc.dram_tensor("mt_all", [P, 2 * TG * P], mdt, kind="ExternalInput")
    wmats = nc.dram_tensor("wmats", [NG, 3, P, P], bdt, kind="ExternalInput")
    ident_in = nc.dram_tensor("ident_in", [P, P], f32, kind="ExternalInput")
    bias_c = nc.dram_tensor("bias_c", [P, NG], f32, kind="ExternalInput")
    wc2_c = nc.dram_tensor("wc2_c", [P, 1], f32, kind="ExternalInput")
    out_loc = nc.dram_tensor("out_loc", [R_PAD, P], f32, kind="ExternalOutput")

    out_r = out_loc.rearrange("(b p) f -> p b f", p=P)

    with tile.TileContext(nc) as tc:
        with (
            tc.tile_pool(name="const", bufs=1) as const,
            tc.tile_pool(name="pers", bufs=1) as pers,
            tc.tile_pool(name="work", bufs=3) as work,
            tc.tile_pool(name="vpool", bufs=16) as vpool,
            tc.tile_pool(name="mtpool", bufs=5) as mtpool,
            tc.tile_pool(name="ppool", bufs=2, space="PSUM") as ppool,
            tc.tile_pool(name="tpsum", bufs=2, space="PSUM") as tpsum,
            tc.tile_pool(name="gpsum", bufs=3, space="PSUM") as gpsum,
            tc.tile_pool(name="xpsum", bufs=1, space="PSUM") as xpsum,
            tc.tile_pool(name="dram", bufs=1, space="DRAM") as dram,
        ):
            # ---------------- latency-critical input DMAs first ----------
            GH = [int(G[:, 0].sum()), int(G[:, 1].sum())]
            # per half: a small head tile (first A_CALLS gather calls) so the
            # first gather fires without waiting for the full index load
            A_COLS = 4 * CALL_G * 8           # 4 calls worth of idx columns
            idx_sb = []
            for h in (0, 1):
                off = 0 if h == 0 else GH[0] * 8
                ca = min(A_COLS, GH[h] * 8)
                ta = pers.tile([P, ca], mybir.dt.int16, tag=f"idxa{h}")
                eng = nc.sync if h == 0 else nc.scalar
                eng.dma_start(out=ta[:], in_=idx_all[:, off:off + ca])
                tb = None
                if GH[h] * 8 > ca:
                    tb = pers.tile([P, GH[h] * 8 - ca], mybir.dt.int16,
                                   tag=f"idxb{h}")
                    nc.scalar.dma_start(out=tb[:],
                                        in_=idx_all[:, off + ca:off + GH[h] * 8])
                idx_sb.append((ta, tb, ca))
            dis = const.tile([P, RB], f32)
            nc.sync.dma_start(out=dis[:], in_=dis_in[:])

            dis2 = const.tile([P, RB], f32)
            nc.vector.tensor_tensor(out=dis2[:], in0=dis[:], in1=dis[:], op=Alu.mult)
            ndis = const.tile([P, RB], f32)
            nc.vector.tensor_scalar(out=ndis[:], in0=dis[:], scalar1=-1.0,
                                    scalar2=None, op0=Alu.mult)
            dis2x = const.tile([P, RB], f32)
            nc.vector.tensor_scalar(out=dis2x[:], in0=dis[:], scalar1=2.0,
                                    scalar2=None, op0=Alu.mult)

            # ---------------- constants ----------------
            ident = const.tile([P, P], f32)
            nc.sync.dma_start(out=ident[:], in_=ident_in[:])
            ident_b = const.tile([P, P], bdt)
            nc.scalar.copy(out=ident_b[:], in_=ident[:])
            wsb = {}
            for gi in range(NG):
                for k in range(3):
                    t = const.tile([P, P], bdt, tag=f"w{gi}{k}")
                    nc.sync.dma_start(out=t[:], in_=wmats[gi, k])
                    wsb[(gi, k)] = t
            bias_sb = const.tile([P, NG], f32)
            nc.sync.dma_start(out=bias_sb[:], in_=bias_c[:])
            wc2_sb = const.tile([P, 1], f32)
            nc.sync.dma_start(out=wc2_sb[:], in_=wc2_c[:])

            # shared SpMM: per half, one contiguous run of gather calls
            # (CALL_G*128 idxs each, SWDGE ring limit) decoupled from block
            # boundaries; per (block, half) a one-hot matmul chain into PSUM.
            qctr = [0]
            nreg_cache = {}

            def nreg(n):
                if n not in nreg_cache:
                    nreg_cache[n] = nc.gpsimd.to_reg(n)
                return nreg_cache[n]

            cumG = np.concatenate([np.zeros((1, 2), np.int64),
                                   np.cumsum(G, axis=0)], axis=0)  # [RB+1, 2]

            def spmm(srcs, moff, consume):
                for h in (0, 1):
                    hoff = 0 if h == 0 else GH[0]
                    nh = GH[h]
                    src_ap = srcs[h]
                    vt = {}
                    emitted = [-1]

                    def ensure_call(k, h=h, hoff=hoff, nh=nh, src_ap=src_ap,
                                    vt=vt, emitted=emitted):
                        while emitted[0] < k:
                            kk = emitted[0] + 1
                            gc = min(CALL_G, nh - kk * CALL_G)
                            c0 = kk * CALL_G * 8          # idx column offset
                            ta, tb, ca = idx_sb[h]
                            if c0 < ca:
                                iap = ta[:, c0:c0 + gc * 8]
                            else:
                                iap = tb[:, c0 - ca:c0 - ca + gc * 8]
                            v = vpool.tile([P, CALL_G, P], edt, tag="v",
                                           name=f"v_{h}_{kk}")
                            nc.gpsimd.dma_gather(
                                out_ap=v[:, :gc, :],
                                in_ap=src_ap,
                                idxs_ap=iap,
                                num_idxs=gc * P, num_idxs_reg=nreg(gc * P),
                                elem_size=P, queue_num=qctr[0] % 4,
                                single_packet=False)
                            qctr[0] += 1
                            vt[kk] = v
                            vt.pop(kk - 16, None)
                            emitted[0] = kk
                    for b in range(RB):
                        s_b, e_b = int(cumG[b, h]), int(cumG[b + 1, h])
                        gs = e_b - s_b
                        goff = hoff + s_b
                        mt = mtpool.tile([P, int(G.max()) * P], mdt, tag="mt")
                        nc.sync.dma_start(
                            out=mt[:, :gs * P],
                            in_=mt_all[:, moff + goff * P:moff + (goff + gs) * P])
                        ps = ppool.tile([P, P], f32, tag="u", name=f"ps_{h}_{b}")
                        for gl_ in range(s_b, e_b):
                            k = gl_ // CALL_G
                            ensure_call(min(k + 5, (nh - 1) // CALL_G))
                            nc.tensor.matmul(
                                out=ps[:], lhsT=mt[:, (gl_ - s_b) * P:(gl_ - s_b + 1) * P],
                                rhs=vt[k][:, gl_ % CALL_G, :],
                                start=(gl_ == s_b), stop=(h == 0 and gl_ == e_b - 1))
                        if h == 1:
                            # fold the h0 partial into the PSUM chain on the
                            # PE (the vector engine is port-starved here)
                            nc.tensor.matmul(out=ps[:], lhsT=ident[:],
                                             rhs=usb[:, b, :],
                                             start=False, stop=True)
                        consume(b, ps, h)

            # ---------------- SpMM 1 (gathers straight from x_tab) --------
            usb = pers.tile([P, RB, P], f32, tag="usb")     # h0 scratch, reused
            u1t = pers.tile([P, RB, P], bdt, tag="u1t")     # Tx1, feature-major
            y2sb = pers.tile([P, RB, P], edt, tag="y2sb")   # dis^2*U1 staging
            # X^T built by per-block PE transposes during SpMM-1 (an xbar
            # transpose-DMA would serialize against the gather DMAs)
            xn_sb = pers.tile([P, RB, P], bdt, tag="xn")
            nc.sync.dma_start(out=xn_sb[:],
                              in_=x_bf.rearrange("(b p) f -> p b f", p=P))
            xt_sb = pers.tile([P, RB, P], bdt, tag="xt")

            y2f = [dram.tile([HALFS[0], P], edt, addr_space="Shared",
                             name="y2f0"),
                   dram.tile([HALFS[1], P], edt, addr_space="Shared",
                             name="y2f1")]
            y2ag_in = [dram.tile([S0B * P, P], edt, name="y2ag0"),
                       dram.tile([S1B * P, P], edt, name="y2ag1")]

            def fire_ag_dma(s):
                b0 = 0 if s == 0 else S0B
                nb = S0B if s == 0 else S1B
                nc.sync.dma_start(
                    out=y2ag_in[s][:].rearrange("(b p) f -> p b f", p=P),
                    in_=y2sb[:, b0:b0 + nb, :])

            def fire_ag_coll(s):
                nc.gpsimd.collective_compute(
                    "AllGather", Alu.bypass,
                    replica_groups=[list(range(NCORES))],
                    ins=[y2ag_in[s].opt()], outs=[y2f[s].opt()])

            # software pipeline state: per-block tiles finished one block late
            pend1 = {}

            def flush1():
                if not pend1:
                    return
                b, at = pend1.popitem()
                tp = tpsum.tile([P, P], f32, tag="tp", space="PSUM")
                nc.tensor.transpose(out=tp[:], in_=at[:], identity=ident[:])
                nc.scalar.copy(out=u1t[:, b, :], in_=tp[:])

            def consume1(b, ps, h):
                if h == 0:
                    tp = xpsum.tile([P, P], bdt, tag="tpx", space="PSUM")
                    nc.tensor.transpose(out=tp[:], in_=xn_sb[:, b, :],
                                        identity=ident_b[:])
                    nc.scalar.copy(out=xt_sb[:, b, :], in_=tp[:])
                    nc.scalar.copy(out=usb[:, b, :], in_=ps[:])
                    return
                nc.scalar.activation(out=y2sb[:, b, :], in_=ps[:], func=Act.Copy,
                                     scale=dis2[:, b:b + 1])
                at = work.tile([P, P], f32, tag="at")
                nc.scalar.activation(out=at[:], in_=ps[:], func=Act.Copy,
                                     scale=ndis[:, b:b + 1])
                flush1()
                pend1[b] = at
                if b == S0B - 1:
                    fire_ag_dma(0)
                elif b == S0B + 3:
                    fire_ag_coll(0)

            spmm((x_tab[0:HALFS[0], :], x_tab[HALFS[0]:NTAB, :]), 0, consume1)
            flush1()
            fire_ag_dma(1)

            # ---------------- SpMM 2 + gates (feature-major), fused -------
            # three-stage software pipeline so every PE item's inputs are at
            # least one full block old: block b's V-ops now; B-transpose for
            # b-1; gate matmuls + activations for b-2; output store for b-3.
            pend_tp = {}
            pend_g = {}
            pend_o = {}

            def flush_out():
                if not pend_o:
                    return
                b, res = pend_o.popitem()
                tpo = tpsum.tile([P, P], f32, tag="tp", space="PSUM")
                nc.tensor.transpose(out=tpo[:], in_=res[:], identity=ident[:])
                res_nm = work.tile([P, P], f32, tag="rnm")
                nc.scalar.copy(out=res_nm[:], in_=tpo[:])
                nc.sync.dma_start(out=out_r[:, b, :], in_=res_nm[:])

            def flush_tp():
                if not pend_tp:
                    return
                b, btmp = pend_tp.popitem()
                tpb = tpsum.tile([P, P], f32, tag="tp", space="PSUM")
                nc.tensor.transpose(out=tpb[:], in_=btmp[:], identity=ident[:])
                btm = work.tile([P, P], bdt, tag="btm")
                nc.scalar.copy(out=btm[:], in_=tpb[:])
                pend_g[b] = btm

            def flush_gates():
                if not pend_g:
                    return
                b, btm = pend_g.popitem()
                gate_ps = []
                for gi in range(NG):
                    pg = gpsum.tile([P, P], f32, tag="g", space="PSUM")
                    nc.tensor.matmul(out=pg[:], lhsT=wsb[(gi, 0)][:],
                                     rhs=xt_sb[:, b, :], start=True, stop=False)
                    nc.tensor.matmul(out=pg[:], lhsT=wsb[(gi, 1)][:],
                                     rhs=u1t[:, b, :], start=False, stop=False)
                    nc.tensor.matmul(out=pg[:], lhsT=wsb[(gi, 2)][:],
                                     rhs=btm[:], start=False, stop=True)
                    gate_ps.append(pg)
                i_t = work.tile([P, P], f32, tag="i")
                nc.scalar.activation(out=i_t[:], in_=gate_ps[0][:], func=Act.Sigmoid,
                                     bias=bias_sb[:, 0:1])
                tt_t = work.tile([P, P], f32, tag="tt")
                nc.scalar.activation(out=tt_t[:], in_=gate_ps[1][:], func=Act.Tanh,
                                     bias=bias_sb[:, 1:2])
                c_t = work.tile([P, P], f32, tag="c")
                nc.vector.tensor_tensor(out=c_t[:], in0=i_t[:], in1=tt_t[:],
                                        op=Alu.mult)
                oin = work.tile([P, P], f32, tag="oin")
                nc.scalar.activation(out=oin[:], in_=c_t[:], func=Act.Copy,
                                     scale=wc2_sb[:, 0:1])
                oin2 = work.tile([P, P], f32, tag="oin2")
                nc.vector.tensor_tensor(out=oin2[:], in0=gate_ps[2][:], in1=oin[:],
                                        op=Alu.add)
                o_t = work.tile([P, P], f32, tag="o")
                nc.scalar.activation(out=o_t[:], in_=oin2[:], func=Act.Sigmoid,
                                     bias=bias_sb[:, 2:3])
                tc_t = work.tile([P, P], f32, tag="tc")
                nc.scalar.activation(out=tc_t[:], in_=c_t[:], func=Act.Tanh)
                h_t = work.tile([P, P], f32, tag="h")
                nc.vector.tensor_tensor(out=h_t[:], in0=o_t[:], in1=tc_t[:],
                                        op=Alu.mult)
                res = work.tile([P, P], f32, tag="res")
                nc.scalar.activation(out=res[:], in_=h_t[:], func=Act.Relu)
                pend_o[b] = res

            def consume2(b, ps2, h):
                if h == 0:
                    if b == 2:
                        fire_ag_coll(1)
                    nc.scalar.copy(out=usb[:, b, :], in_=ps2[:])
                    return
                btmp = work.tile([P, P], f32, tag="btmp")
                nc.scalar.activation(out=btmp[:], in_=ps2[:], func=Act.Copy,
                                     scale=dis2x[:, b:b + 1])
                flush_out()
                flush_gates()
                flush_tp()
                pend_tp[b] = btmp

            spmm((y2f[0][:], y2f[1][:]), TG * P, consume2)
            while pend_tp or pend_g or pend_o:
                flush_out()
                flush_tp()
                flush_gates()

    nc.compile()
    return nc


# ----------------------------------------------------------------------------
# Entry point
# ----------------------------------------------------------------------------

_CACHE = {}


def _get_built(cfg_key, cfg):
    if cfg_key not in _CACHE:
        _CACHE[cfg_key] = _build(cfg)
    return _CACHE[cfg_key]


def _make_in_maps(inputs):
    node_feats = np.asarray(inputs["node_feats"])
    edge_feats = np.asarray(inputs["edge_feats"], np.float32)
    edge_index = np.asarray(inputs["edge_index"])
    t = node_feats.shape[0] - 1
    X = np.asarray(node_feats[t], np.float32)
    row = np.asarray(edge_index[t, 0], np.int64)
    col = np.asarray(edge_index[t, 1], np.int64)
    w = np.asarray(edge_feats[t], np.float32)

    in_maps, cfg = _preprocess(X, row, col, w)

    Wx = np.asarray(inputs["Wx"], np.float32)
    bsum = (np.asarray(inputs["bx"], np.float32)
            + np.asarray(inputs["bh"], np.float32)
            + np.asarray(inputs["bg"], np.float32))          # [4, FH]
    wc = np.asarray(inputs["wc"], np.float32)                # [3, FH]
    GATES = (0, 2, 3)
    wm = np.empty((3, 3, P, P), BF16)
    for gi, g in enumerate(GATES):
        wm[gi, 0] = (Wx[g, 0] - Wx[g, 2]).astype(BF16)
        wm[gi, 1] = Wx[g, 1].astype(BF16)
        wm[gi, 2] = Wx[g, 2].astype(BF16)
    bias_c = np.ascontiguousarray(bsum[list(GATES)].T)       # [P, 3]
    wc2_c = np.ascontiguousarray(wc[2].reshape(P, 1))        # [P, 1]
    ident = np.eye(P, dtype=np.float32)
    for m in in_maps:
        m["wmats"] = wm
        m["bias_c"] = bias_c
        m["wc2_c"] = wc2_c
        m["ident_in"] = ident
    return in_maps, cfg


def _run(inputs, trace=False):
    from concourse.bass_utils import run_bass_kernel_spmd

    in_maps, cfg = _make_in_maps(inputs)
    key = (cfg["N"], cfg["RB"], cfg["TG"],
           tuple(cfg["G"].ravel().tolist()))
    nc = _get_built(key, cfg)
    res = run_bass_kernel_spmd(nc, in_maps, core_ids=list(range(NCORES)),
                               trace=trace)
    N, R, R_PAD = cfg["N"], cfg["R"], cfg["R_PAD"]
    out = np.empty((N, P), np.float32)
    for c in range(NCORES):
        lo, hi = c * R, min((c + 1) * R, N)
        out[lo:hi] = res.results[c]["out_loc"][: hi - lo]
    return out, res.exec_time_ns


def kernel(**inputs) -> np.ndarray:
    out, _ = _run(inputs, trace=False)
    return out
